# revision 4
# baseline (speedup 1.0000x reference)
"""Trainium2 Bass kernel for nn_AttentionWithContext.

Reference computation (per example b):
    key    = x @ W + b                      # [T, C]
    logits = (key . u) / sqrt(D)            # [T]
    softmax over T with padding mask (masked -> row min, shift by max)
    scores = mask * exp(logits - max) / sum # [T]
    output = scores @ x                     # [D]

Key algebraic collapse: logits = x @ v + c with
    v = (W @ u) / sqrt(D)   (precomputed on host in float64)
    c = (b . u) / sqrt(D)
so the [B,T,C] Dense activation never needs to be materialized; the kernel is
memory-bound on a single read of x.

Sharding: batch (64) split across 8 NeuronCores, 8 examples per core,
replicated v. No collectives.

Per-core layout: example x_e [2048, 256] lives in one SBUF tile [128, 4096]
with token t = p*16 + n (partition p, slot n) so every partition reads one
contiguous 16 KiB block from HBM.

Engine split per example:
  - logits: DVE tensor_tensor_reduce (fused mul+row-reduce) for K_TTR
    examples; DVE product + ScalarE Identity-with-accum reduce for the rest.
  - masked max: DVE free-dim max -> GPSIMD partition all-reduce(max)
  - exp + row sum fused on ScalarE (bias = -max per partition)
  - denominator: GPSIMD partition all-reduce(add), DVE reciprocal
  - scores = w * (1/S) on ScalarE; pooling on PE: x chunks [128,128] as
    stationary, score column [128,1] moving, PSUM-accumulated over 16 tiles.
"""

import os

os.environ.setdefault("MYCRO_LOCAL_CACHE", "1")

from contextlib import ExitStack

import numpy as np

B, T, D = 64, 2048, 256
N_CORES = 8
BPC = B // N_CORES  # examples per core
P = 128  # partitions
NT = T // P  # 16 token slots per partition
K_TTR = 3  # examples whose logits reduce on DVE (rest reduce on ScalarE)
BIG = 1.0e30


def build_body(ctx, tc, outs, ins):
    """Emit the per-core program. ins = (x, mask, vb); outs = (scores, pooled).

    x      [BPC, T, D]  fp32  ExternalInput
    mask   [BPC, T]     fp32  ExternalInput
    vb     [P, D]       fp32  ExternalInput (v broadcast to all partitions)
    scores [BPC, T]     fp32  ExternalOutput
    pooled [P, 2*BPC]   fp32  ExternalOutput; pooled[p, 2*e+ch] = out[e, ch*128+p]
    """
    import concourse.mybir as mybir
    from concourse import bass_isa

    F32 = mybir.dt.float32
    AX = mybir.AxisListType
    OP = mybir.AluOpType
    AF = mybir.ActivationFunctionType

    nc = tc.nc
    x, mask, vb = ins
    scores_out, pooled_out = outs

    const_pool = ctx.enter_context(tc.tile_pool(name="const", bufs=1))
    x_pool = ctx.enter_context(tc.tile_pool(name="xres", bufs=3))
    prod_pool = ctx.enter_context(tc.tile_pool(name="prod", bufs=2))
    trash_pool = ctx.enter_context(tc.tile_pool(name="trash", bufs=3))
    small_pool = ctx.enter_context(tc.tile_pool(name="small", bufs=4))
    col_pool = ctx.enter_context(tc.tile_pool(name="col", bufs=8))
    psum_pool = ctx.enter_context(tc.tile_pool(name="psum", bufs=4, space="PSUM"))

    # --- constants / staging ---
    v_t = const_pool.tile([P, D], F32, name="v_t")
    nc.sync.dma_start(v_t[:], vb)

    mask_all = const_pool.tile([P, BPC * NT], F32, name="mask_all")
    nc.sync.dma_start(
        mask_all[:].rearrange("p (e n) -> p e n", e=BPC),
        mask.rearrange("e (p n) -> p e n", p=P),
    )
    # penalty = (mask - 1) * BIG  -> 0 where valid, -BIG where masked
    pen_all = const_pool.tile([P, BPC * NT], F32, name="pen_all")
    nc.vector.tensor_scalar(pen_all[:], mask_all[:], -1.0, BIG, OP.add, OP.mult)

    scores_all = const_pool.tile([P, BPC * NT], F32, name="scores_all")
    pooled_all = const_pool.tile([P, 2 * BPC], F32, name="pooled_all")

    for e in range(BPC):
        x_t = x_pool.tile([P, NT * D], F32, name="x_t")
        nc.sync.dma_start(x_t[:], x[e].rearrange("(p n) d -> p (n d)", p=P))

        # --- stage 1: logits[p, n] = sum_d x[p, n, d] * v[d] ---
        logits = small_pool.tile([P, NT], F32, name="logits")
        if e < K_TTR:
            for n in range(NT):
                trash = trash_pool.tile([P, D], F32, name="trash")
                nc.vector.affine_mul_reduce(
                    out=trash[:],
                    accum_out=logits[:, n : n + 1],
                    in0=x_t[:, n * D : (n + 1) * D],
                    in1=v_t[:],
                    scale=1.0,
                    bias=0.0,
                )
        else:
            prod = prod_pool.tile([P, NT * D], F32, name="prod")
            nc.vector.tensor_mul(
                prod[:].rearrange("p (n d) -> p n d", n=NT),
                x_t[:].rearrange("p (n d) -> p n d", n=NT),
                v_t[:].unsqueeze(1).broadcast_to([P, NT, D]),
            )
            for n in range(NT):
                trash = trash_pool.tile([P, D], F32, name="trash")
                nc.scalar.activation(
                    trash[:],
                    prod[:, n * D : (n + 1) * D],
                    AF.Identity,
                    bias=0.0,
                    scale=1.0,
                    accum_out=logits[:, n : n + 1],
                )

        # --- stage 2: masked softmax over all T = (p, n) ---
        mask_blk = mask_all[:, e * NT : (e + 1) * NT]
        pen_blk = pen_all[:, e * NT : (e + 1) * NT]

        lm = small_pool.tile([P, NT], F32, name="lm")
        nc.vector.tensor_mul(lm[:], logits[:], mask_blk)
        masked = small_pool.tile([P, NT], F32, name="masked")
        nc.vector.tensor_add(masked[:], lm[:], pen_blk)

        mp = col_pool.tile([P, 1], F32, name="mp")
        nc.vector.reduce_max(mp[:], masked[:], axis=AX.X)
        m_all = col_pool.tile([P, 1], F32, name="m_all")
        nc.gpsimd.partition_all_reduce(
            m_all[:], mp[:], channels=P, reduce_op=bass_isa.ReduceOp.max
        )
        negm = col_pool.tile([P, 1], F32, name="negm")
        nc.scalar.mul(negm[:], m_all[:], -1.0)

        # w = exp(masked - M); masked slots underflow to exactly 0
        w_blk = small_pool.tile([P, NT], F32, name="w_blk")
        sp = col_pool.tile([P, 1], F32, name="sp")
        nc.scalar.activation(
            w_blk[:], masked[:], AF.Exp, bias=negm[:], scale=1.0, accum_out=sp[:]
        )
        s_all = col_pool.tile([P, 1], F32, name="s_all")
        nc.gpsimd.partition_all_reduce(
            s_all[:], sp[:], channels=P, reduce_op=bass_isa.ReduceOp.add
        )
        rs = col_pool.tile([P, 1], F32, name="rs")
        nc.vector.reciprocal(rs[:], s_all[:])

        sc_blk = scores_all[:, e * NT : (e + 1) * NT]
        nc.scalar.mul(sc_blk, w_blk[:], rs[:])
        nc.sync.dma_start(scores_out[e].rearrange("(p n) -> p n", p=P), sc_blk)

        # --- pass 2: pooled[d] = sum_t scores[t] * x[t, d] on PE ---
        pooled_ps = psum_pool.tile([P, 2], F32, name="pooled_ps")
        for ch in range(2):
            for n in range(NT):
                nc.tensor.matmul(
                    out=pooled_ps[:, ch : ch + 1],
                    lhsT=x_t[:, n * D + ch * P : n * D + ch * P + P],
                    rhs=sc_blk[:, n : n + 1],
                    start=(n == 0),
                    stop=(n == NT - 1),
                )
        nc.scalar.copy(pooled_all[:, 2 * e : 2 * e + 2], pooled_ps[:])

    nc.sync.dma_start(pooled_out, pooled_all[:])


def _declare_io(nc):
    import concourse.mybir as mybir

    F32 = mybir.dt.float32
    x = nc.dram_tensor("x", [BPC, T, D], F32, kind="ExternalInput").ap()
    mask = nc.dram_tensor("mask", [BPC, T], F32, kind="ExternalInput").ap()
    vb = nc.dram_tensor("vb", [P, D], F32, kind="ExternalInput").ap()
    scores = nc.dram_tensor("scores", [BPC, T], F32, kind="ExternalOutput").ap()
    pooled = nc.dram_tensor("pooled", [P, 2 * BPC], F32, kind="ExternalOutput").ap()
    return (x, mask, vb), (scores, pooled)


_NC_CACHE = None


def _build_nc():
    global _NC_CACHE
    if _NC_CACHE is not None:
        return _NC_CACHE
    import concourse.bacc as bacc
    import concourse.tile as tile

    nc = bacc.Bacc(
        "TRN2",
        target_bir_lowering=False,
        debug=False,
        enable_asserts=True,
        num_devices=N_CORES,
    )
    ins, outs = _declare_io(nc)
    with tile.TileContext(nc) as tc:
        with ExitStack() as ctx:
            build_body(ctx, tc, outs, ins)
    nc.compile()
    _NC_CACHE = nc
    return nc


def host_prep(inputs, mask, W, b, u):
    """Host-side: fold W/b/u into the logit vector v and constant c."""
    v = (W.astype(np.float64) @ u.astype(np.float64)) / np.sqrt(np.float64(D))
    c = float(b.astype(np.float64) @ u.astype(np.float64) / np.sqrt(np.float64(D)))
    assert c == 0.0, "b is expected to be zeros; c-fold not emitted in kernel"
    v32 = v.astype(np.float32)
    vb = np.ascontiguousarray(np.broadcast_to(v32[None, :], (P, D)))
    return vb


def run(inputs, mask, W, b, u, **spmd_kwargs):
    from concourse.bass_utils import run_bass_kernel_spmd

    inputs = np.ascontiguousarray(inputs, dtype=np.float32)
    mask = np.ascontiguousarray(mask, dtype=np.float32)
    vb = host_prep(inputs, mask, W, b, u)

    nc = _build_nc()
    in_maps = []
    for i in range(N_CORES):
        sl = slice(i * BPC, (i + 1) * BPC)
        in_maps.append(
            {
                "x": np.ascontiguousarray(inputs[sl]),
                "mask": np.ascontiguousarray(mask[sl]),
                "vb": vb,
            }
        )
    res = run_bass_kernel_spmd(nc, in_maps, core_ids=list(range(N_CORES)), **spmd_kwargs)

    scores = np.empty((B, T), dtype=np.float32)
    output = np.empty((B, D), dtype=np.float32)
    for i in range(N_CORES):
        r = res.results[i]
        scores[i * BPC : (i + 1) * BPC] = r["scores"]
        # pooled[p, 2e+ch] = out[e, ch*128+p]
        pooled = r["pooled"].reshape(P, BPC, 2).transpose(1, 2, 0).reshape(BPC, D)
        output[i * BPC : (i + 1) * BPC] = pooled
    return (output, scores), res


def kernel(inputs, mask, W, b, u):
    (output, scores), _ = run(inputs, mask, W, b, u)
    return output, scores
